# revision 26
# baseline (speedup 1.0000x reference)
"""EGCL (E(n)-equivariant GNN layer) Trainium2 kernel.

Strategy (8 NeuronCores, data-parallel, no collectives):
  - Sort edges by destination node (row); node space padded to 392 windows of
    128 nodes; each core owns 49 consecutive windows (6272 nodes) and every
    edge landing in them.
  - Separable first layer: e_in @ We1 = p[row] + q[col] + radial*w_r with
    p = h@We1a, q = h@We1b computed per-node on device (phase A); q is stored
    bf16 in DRAM and randomly gathered per edge with a transposing dma_gather;
    p is gathered from resident SBUF with the SBUF-source dma_gather.
  - Edge MLP runs in "T layout" (features on partitions, edges on the free
    axis) so weight matmuls keep weights stationary; per 128-edge tile a
    one-hot matrix (DVE compare vs rowrel) is the stationary operand of the
    segment-sum scatter which accumulates agg/force/cnt in PSUM per window.
  - Node MLP, force normalization and the velocity head run per window on
    resident hT/agg tiles; outputs are written in blocked/transposed layouts
    the host reassembles.
"""

import os
import sys

import numpy as np

for _p in ("/opt/trn_rl_repo",):
    if os.path.isdir(_p) and _p not in sys.path:
        sys.path.insert(0, _p)

import ml_dtypes  # noqa: E402

# ---------------- problem geometry (hardcoded) ----------------
N, E, D, H = 50000, 800000, 128, 128
NCORES = 8
NW = 392                 # total node windows (392*128 = 50176 >= N)
WPC = NW // NCORES       # 49 windows per core
NPC = WPC * 128          # 6272 nodes per core
NPAD = NW * 128          # 50176
NHALF = NPAD // 2        # 25088 (dma_gather int16 index range split)
CH_T = 6                 # tiles (of 128 edges) per pipeline chunk

USE_ACT_SILU = True      # Silu on ACT; False -> Sigmoid + DVE multiply
CAP_WAITS = True         # walrus workaround; CoreSim needs False


# ---------------- host-side preprocessing ----------------

def _prep(h, coord_diff, row, col):
    """Sort/pad edges into window-major device layouts."""
    row = np.asarray(row).astype(np.int64)
    col = np.asarray(col).astype(np.int64)
    cd = np.asarray(coord_diff, dtype=np.float32)

    key = row >> 7
    order = np.argsort(key, kind="stable")
    row_s, col_s, cd_s = row[order], col[order], cd[order]

    counts = np.bincount(key, minlength=NW)
    caph = int(-(-counts.max() // 768) * 768)   # CAP multiple of 768 (CH_T=6)
    cap = caph
    T = cap // 128

    rowrel = np.full((NW, cap), -1.0, np.float32)
    colidx = np.zeros((NW, cap), np.int32)
    cdp = np.zeros((NW, cap, 3), np.float32)
    starts = np.zeros(NW + 1, np.int64)
    starts[1:] = np.cumsum(counts)
    for w in range(NW):
        s, e = starts[w], starts[w + 1]
        n = e - s
        if n == 0:
            continue
        rowrel[w, :n] = (row_s[s:e] - w * 128).astype(np.float32)
        colidx[w, :n] = col_s[s:e].astype(np.int32)
        cdp[w, :n] = cd_s[s:e]

    # device layouts; tile t = pos // 128, partition = pos % 128
    rr_dev = (rowrel.reshape(NW, T, 128).transpose(0, 2, 1)
              .astype(ml_dtypes.bfloat16).copy())                     # [NW,128,T]
    rrT_dev = rowrel.reshape(NW, 1, cap).copy()                       # [NW,1,CAP] f32
    ci_dev = colidx.reshape(NW, T, 128).transpose(0, 2, 1).copy()     # [NW,128,T]
    cd_dev = (cdp.reshape(NW, T, 128, 3)
              .transpose(0, 2, 1, 3).reshape(NW, 128, T * 3).copy())  # f32
    return rr_dev, rrT_dev, ci_dev, cd_dev, T


def _weights_np(ws, T):
    f32 = np.float32
    b16 = ml_dtypes.bfloat16
    We1 = np.asarray(ws["We1"], f32)
    out = {
        "We1a": We1[:128].copy(),
        "We1b": We1[128:256].copy(),
        "wr": We1[256:257].astype(b16),
        "We2": np.asarray(ws["We2"], f32).astype(b16),
        "Wc1": np.asarray(ws["Wc1"], f32).astype(b16),
        "Wc2": np.asarray(ws["Wc2"], f32).astype(b16),
        "Wn1a": np.asarray(ws["Wn1"], f32)[:128].copy(),
        "Wn1b": np.asarray(ws["Wn1"], f32)[128:256].copy(),
        "Wn2": np.asarray(ws["Wn2"], f32),
        "Wv1": np.asarray(ws["Wv1"], f32),
        "Wv2": np.asarray(ws["Wv2"], f32),
        "be1": np.asarray(ws["be1"], f32).reshape(128, 1),
        "be2": np.asarray(ws["be2"], f32).reshape(128, 1),
        "bc1": np.asarray(ws["bc1"], f32).reshape(128, 1),
        "bn1": np.asarray(ws["bn1"], f32).reshape(128, 1),
        "bn2": np.asarray(ws["bn2"], f32).reshape(128, 1),
        "bv1": np.asarray(ws["bv1"], f32).reshape(128, 1),
        "bv2": np.asarray(ws["bv2"], f32).reshape(1, 1),
        "iotat": np.tile(np.arange(128, dtype=f32), (128, T)).astype(b16),
        "iotap": np.arange(128, dtype=f32).reshape(128, 1),
        "ones1": np.ones((1, 128), f32),
        "idf": np.eye(128, dtype=f32),
        "idb": np.eye(128, dtype=f32).astype(b16),
    }
    return out


# ---------------- device program ----------------

def _build(T):
    import concourse.bass as bass
    import concourse.mybir as mybir
    import concourse.tile as tile
    from concourse import library_config
    from concourse.tile_rust import add_dep_helper
    from contextlib import ExitStack

    dt = mybir.dt
    f32, b16, i16 = dt.float32, dt.bfloat16, dt.int16
    AF = mybir.ActivationFunctionType
    OP = mybir.AluOpType

    CAP = T * 128
    CAPH = CAP // 2
    C16 = CAPH // 16
    CH = CH_T * 128                       # chunk edge count (768)
    n_chunks = CAP // CH

    nc = bass.Bass("TRN2", target_bir_lowering=False, debug=False,
                   num_devices=NCORES)

    IN = {}

    def inp(name, shape, dtype):
        IN[name] = nc.dram_tensor(name, list(shape), dtype, kind="ExternalInput")
        return IN[name]

    inp("hT", [128, NPAD], f32)
    inp("hT_own", [128, NPC], f32)
    inp("rr", [WPC, 128, T], b16)
    inp("rrT", [WPC, 1, CAP], f32)
    inp("cdp", [WPC, 128, T * 3], f32)
    inp("ci", [WPC, 128, T], dt.int32)
    inp("We1a", [128, H], f32); inp("We1b", [128, H], f32)
    inp("wr", [1, H], b16)
    inp("We2", [H, H], b16); inp("Wc1", [H, H], b16); inp("Wc2", [H, 1], b16)
    inp("Wn1a", [128, H], f32); inp("Wn1b", [H, H], f32); inp("Wn2", [H, D], f32)
    inp("Wv1", [128, H], f32); inp("Wv2", [H, 1], f32)
    for b in ("be1", "be2", "bc1", "bn1", "bn2", "bv1"):
        inp(b, [128, 1], f32)
    inp("bv2", [1, 1], f32)
    inp("iotat", [128, CAP], b16)
    inp("iotap", [128, 1], f32)
    inp("ones1", [1, 128], f32)
    inp("idf", [128, 128], f32)
    inp("idb", [128, 128], b16)

    houtT = nc.dram_tensor("houtT", [WPC, 128, 128], f32, kind="ExternalOutput")
    force_o = nc.dram_tensor("force_o", [128, WPC * 3], f32, kind="ExternalOutput")
    vel_o = nc.dram_tensor("vel_o", [1, WPC * 128], f32, kind="ExternalOutput")

    qd = nc.dram_tensor("qd", [NPAD, H], b16)

    with tile.TileContext(nc) as tc, ExitStack() as ctx:
        cpool = ctx.enter_context(tc.tile_pool(name="const", bufs=1))
        io = ctx.enter_context(tc.tile_pool(name="io", bufs=4))
        wk = ctx.enter_context(tc.tile_pool(name="wk", bufs=2))
        ps = ctx.enter_context(
            tc.tile_pool(name="ps", bufs=3, space=bass.MemorySpace.PSUM))
        psb = ctx.enter_context(
            tc.tile_pool(name="psb", bufs=1, space=bass.MemorySpace.PSUM))
        psP = ctx.enter_context(
            tc.tile_pool(name="psP", bufs=1, space=bass.MemorySpace.PSUM))

        def mm_split(out, lhsT, rhs, n):
            # matmul in <=512-column pieces so each lands in one PSUM bank
            for s0 in range(0, n, 512):
                s1 = min(s0 + 512, n)
                nc.tensor.matmul(out[:, s0:s1], lhsT, rhs[:, s0:s1],
                                 start=True, stop=True)

        # ---- constants / persistent SBUF ----
        C = {}
        for name in ("We1a", "We1b", "wr", "We2", "Wc1", "Wc2", "Wn1a", "Wn1b",
                     "Wn2", "Wv1", "Wv2", "be1", "be2", "bc1", "bn1", "bn2",
                     "bv1", "bv2", "iotat", "iotap", "ones1", "idf", "idb"):
            t = cpool.tile(list(IN[name].shape), IN[name].dtype, tag=name)
            nc.sync.dma_start(t[:], IN[name].ap())
            C[name] = t

        hT_res = cpool.tile([128, NPC], f32, tag="hT_res")
        p_res = cpool.tile([128, NPC], b16, tag="p_res")
        force_acc = cpool.tile([128, WPC * 3], f32, tag="force_acc")
        vel_acc = cpool.tile([1, WPC * 128], f32, tag="vel_acc")

        nc.sync.dma_start(hT_res[:], IN["hT_own"].ap())

        # ---- phase A: p (bf16, resident) for own windows; q (bf16) in DRAM --
        for w in range(WPC):
            pp = ps.tile([128, CH], f32, tag="ps")
            nc.tensor.matmul(pp[:, :H], hT_res[:, 128 * w:128 * (w + 1)],
                             C["We1a"][:], start=True, stop=True)
            nc.vector.tensor_copy(p_res[:, 128 * w:128 * (w + 1)], pp[:, :H])

        last_q_write = None
        for j in range(NW):
            ht = io.tile([128, 128], f32, tag="ht")
            nc.sync.dma_start(ht[:], IN["hT"].ap()[:, 128 * j:128 * (j + 1)])
            qp = ps.tile([128, CH], f32, tag="ps")
            nc.tensor.matmul(qp[:, :H], ht[:], C["We1b"][:], start=True, stop=True)
            qs = io.tile([128, H], b16, tag="qs")
            nc.vector.tensor_copy(qs[:], qp[:, :H])
            last_q_write = nc.sync.dma_start(
                qd.ap()[128 * j:128 * (j + 1), :], qs[:])

        # ---- phase B/C: per-window edge pipeline + node ops ----
        for w in range(WPC):
            rr_w = wk.tile([128, T], b16, tag="rr")
            nc.sync.dma_start(rr_w[:], IN["rr"].ap()[w])
            cd_w = wk.tile([128, T * 3], f32, tag="cd")
            nc.sync.dma_start(cd_w[:], IN["cdp"].ap()[w])

            rrT_w = wk.tile([1, CAP], f32, tag="rrT")
            nc.sync.dma_start(rrT_w[:], IN["rrT"].ap()[w])
            ci_w = wk.tile([128, T], dt.int32, tag="ci")
            nc.sync.dma_start(ci_w[:], IN["ci"].ap()[w])

            # q[col] gather (e-layout rows, one indirect DMA per 128-edge tile)
            qe_w = wk.tile([128, T * 128], b16, tag="qe")
            for t in range(T):
                g = nc.gpsimd.indirect_dma_start(
                    out=qe_w[:, 128 * t:128 * (t + 1)],
                    out_offset=None,
                    in_=qd.ap(),
                    in_offset=bass.IndirectOffsetOnAxis(
                        ap=ci_w[:, t:t + 1], axis=0),
                )
                if last_q_write is not None:
                    add_dep_helper(g.ins, last_q_write.ins, True,
                                   "gather after q fully written")

            mask_w = wk.tile([128, T], b16, tag="mask")
            nc.vector.tensor_scalar(mask_w[:], rr_w[:], 0.0, None, OP.is_ge)

            # one-hot: ohB[e, n] = (iota_n == rowrel[e])  (single DVE op)
            ohB = wk.tile([128, CAP], b16, tag="ohB")
            nc.vector.scalar_tensor_tensor(
                ohB[:].rearrange("p (t n) -> p t n", n=128),
                C["iotat"][:].rearrange("p (t n) -> p t n", n=128),
                0.0,
                rr_w[:].broadcast_to([128, T, 128]),
                OP.add, OP.is_equal)

            # radial: |cd|^2 -> [1, CAP] bf16 via transpose + sbuf-sbuf dma
            sq_w = wk.tile([128, T * 3], f32, tag="sq")
            nc.vector.tensor_tensor(sq_w[:], cd_w[:], cd_w[:], OP.mult)
            radial_e = wk.tile([128, T], f32, tag="radial")
            nc.vector.tensor_reduce(
                radial_e[:], sq_w[:].rearrange("p (t i) -> p t i", i=3),
                mybir.AxisListType.X, OP.add)
            rT_ps = ps.tile([128, CH], f32, tag="ps")
            nc.tensor.transpose(rT_ps[:T, :128], radial_e[:], C["idf"][:])
            rT_sb = wk.tile([T, 128], b16, tag="rT")
            nc.vector.tensor_copy(rT_sb[:], rT_ps[:T, :128])
            radialT = wk.tile([1, CAP], b16, tag="radialT")
            nc.sync.dma_start(radialT[:], rT_sb[:])

            scat = psP.tile([128, 132], f32, tag="scat")
            z1_w = wk.tile([128, CAP], b16, tag="z1")
            a1T = wk.tile([128, CAP], b16, tag="a1T")
            mT = wk.tile([128, CAP], b16, tag="mT")
            z3_w = wk.tile([128, CAP], b16, tag="z3")
            sT = wk.tile([128, CAP], b16, tag="sT")

            # --- layer 1: z1 = p[row] + q[col] + w_r*radial ; a1 = silu+be1
            for ci in range(n_chunks):
                e0 = ci * CH
                sl = slice(e0, e0 + CH)
                # one-hot A (rowrel broadcast across partitions via ones-matmul)
                rrp = ps.tile([128, CH], f32, tag="ps")
                mm_split(rrp, C["ones1"][:], rrT_w[:, sl], CH)
                ohA = wk.tile([128, CH], b16, tag="ohA")
                nc.vector.tensor_scalar(ohA[:], rrp[:], C["iotap"][:, :1],
                                        None, OP.is_equal)
                m1ps = ps.tile([128, CH], f32, tag="ps")
                for s0 in range(0, CH, 512):
                    s1 = min(s0 + 512, CH)
                    nc.tensor.matmul(m1ps[:, s0:s1], C["wr"][:],
                                     radialT[:, sl][:, s0:s1],
                                     start=True, stop=False)
                    nc.tensor.matmul(m1ps[:, s0:s1],
                                     p_res[:, 128 * w:128 * (w + 1)],
                                     ohA[:, s0:s1], start=False, stop=True)
                qt_ps = psb.tile([128, CH], b16, tag="psb")
                for i in range(CH_T):
                    gt = ci * CH_T + i
                    nc.tensor.transpose(qt_ps[:, 128 * i:128 * (i + 1)],
                                        qe_w[:, 128 * gt:128 * (gt + 1)],
                                        C["idb"][:])
                qt_sb = wk.tile([128, CH], b16, tag="qt")
                nc.vector.tensor_copy(qt_sb[:], qt_ps[:])
                nc.vector.tensor_tensor(z1_w[:, sl], m1ps[:], qt_sb[:], OP.add)
            if USE_ACT_SILU:
                nc.scalar.activation(a1T[:], z1_w[:], AF.Silu, bias=C["be1"][:])
            else:
                sg = wk.tile([128, CAP], b16, tag="sg1")
                nc.scalar.activation(sg[:], z1_w[:], AF.Sigmoid, bias=C["be1"][:])
                nc.vector.scalar_tensor_tensor(
                    a1T[:], z1_w[:], C["be1"][:, :1], sg[:], OP.add, OP.mult)

            # --- layer 2: mT = silu(We2.T@a1 + be2) (chunked from PSUM)
            for ci in range(n_chunks):
                sl = slice(ci * CH, ci * CH + CH)
                m2ps = ps.tile([128, CH], f32, tag="ps")
                mm_split(m2ps, C["We2"][:], a1T[:, sl], CH)
                if USE_ACT_SILU:
                    nc.scalar.activation(mT[:, sl], m2ps[:], AF.Silu,
                                         bias=C["be2"][:])
                else:
                    sg = wk.tile([128, CH], b16, tag="sg2")
                    nc.scalar.activation(sg[:], m2ps[:], AF.Sigmoid,
                                         bias=C["be2"][:])
                    nc.vector.scalar_tensor_tensor(
                        mT[:, sl], m2ps[:], C["be2"][:, :1], sg[:],
                        OP.add, OP.mult)

            # --- coef head: sT = silu(Wc1.T@mT + bc1) (z3 batched)
            for ci in range(n_chunks):
                sl = slice(ci * CH, ci * CH + CH)
                c1ps = ps.tile([128, CH], f32, tag="ps")
                mm_split(c1ps, C["Wc1"][:], mT[:, sl], CH)
                nc.vector.tensor_copy(z3_w[:, sl], c1ps[:])
            if USE_ACT_SILU:
                nc.scalar.activation(sT[:], z3_w[:], AF.Silu, bias=C["bc1"][:])
            else:
                sg = wk.tile([128, CAP], b16, tag="sg3")
                nc.scalar.activation(sg[:], z3_w[:], AF.Sigmoid, bias=C["bc1"][:])
                nc.vector.scalar_tensor_tensor(
                    sT[:], z3_w[:], C["bc1"][:, :1], sg[:], OP.add, OP.mult)

            # --- coef, trans, scatter (per chunk) ---
            for ci in range(n_chunks):
                t0 = ci * CH_T
                cfps = ps.tile([128, CH], f32, tag="ps")
                for i in range(CH_T):
                    gt = t0 + i
                    nc.tensor.matmul(cfps[:, i:i + 1],
                                     sT[:, 128 * gt:128 * (gt + 1)],
                                     C["Wc2"][:], start=True, stop=True)
                coef = wk.tile([128, CH_T], f32, tag="coef")
                nc.vector.tensor_copy(coef[:], cfps[:, :CH_T])

                srhs = wk.tile([128, CH_T * 132], b16, tag="srhs")
                sv = srhs[:].rearrange("p (t c) -> p t c", c=132)
                mtr_ps = psb.tile([128, CH], b16, tag="psb")
                for i in range(CH_T):
                    gt = t0 + i
                    nc.tensor.transpose(mtr_ps[:, 128 * i:128 * (i + 1)],
                                        mT[:, 128 * gt:128 * (gt + 1)],
                                        C["idb"][:])
                nc.vector.tensor_copy(
                    sv[:, :, 0:128],
                    mtr_ps[:].rearrange("p (t e) -> p t e", e=128))
                tmptr = wk.tile([128, CH_T * 3], f32, tag="tmptr")
                nc.vector.tensor_tensor(
                    tmptr[:].rearrange("p (t i) -> p t i", i=3),
                    cd_w[:, 3 * t0:3 * (t0 + CH_T)]
                        .rearrange("p (t i) -> p t i", i=3),
                    coef[:].broadcast_to([128, CH_T, 3]),
                    OP.mult)
                nc.vector.tensor_scalar(
                    sv[:, :, 128:131],
                    tmptr[:].rearrange("p (t i) -> p t i", i=3),
                    -100.0, 100.0, OP.max, OP.min)
                nc.vector.tensor_copy(
                    sv[:, :, 131:132],
                    mask_w[:, t0:t0 + CH_T].broadcast_to([128, CH_T, 1]))

                for i in range(CH_T):
                    gt = t0 + i
                    nc.tensor.matmul(scat[:], ohB[:, 128 * gt:128 * (gt + 1)],
                                     sv[:, i, :], start=(gt == 0),
                                     stop=(gt == T - 1))

            # ---- window epilogue ----
            cntc = wk.tile([128, 1], f32, tag="cntc")
            nc.vector.tensor_scalar(cntc[:], scat[:, 131:132], 1.0, None, OP.max)
            rcp = wk.tile([128, 1], f32, tag="rcp")
            nc.vector.reciprocal(rcp[:], cntc[:])
            nc.vector.tensor_scalar(force_acc[:, 3 * w:3 * (w + 1)],
                                    scat[:, 128:131], rcp[:, :1], None, OP.mult)

            agg_sb = wk.tile([128, 128], f32, tag="agg")
            nc.vector.tensor_copy(agg_sb[:], scat[:, 0:128])
            aggT_ps = ps.tile([128, CH], f32, tag="ps")
            nc.tensor.transpose(aggT_ps[:, :128], agg_sb[:], C["idf"][:])
            aggT = wk.tile([128, 128], f32, tag="aggT")
            nc.vector.tensor_copy(aggT[:], aggT_ps[:, :128])

            # node MLP (T layout): houtT[w] = Wn2.T @ silu(zpre) + bn2
            zps = ps.tile([128, CH], f32, tag="ps")
            nc.tensor.matmul(zps[:, :H], C["Wn1a"][:],
                             hT_res[:, 128 * w:128 * (w + 1)],
                             start=True, stop=False)
            nc.tensor.matmul(zps[:, :H], C["Wn1b"][:], aggT[:],
                             start=False, stop=True)
            zT = wk.tile([128, 128], f32, tag="zT")
            if USE_ACT_SILU:
                nc.scalar.activation(zT[:], zps[:, :H], AF.Silu, bias=C["bn1"][:])
            else:
                sg = wk.tile([128, 128], f32, tag="sgn")
                nc.scalar.activation(sg[:], zps[:, :H], AF.Sigmoid,
                                     bias=C["bn1"][:])
                nc.vector.scalar_tensor_tensor(
                    zT[:], zps[:, :H], C["bn1"][:, :1], sg[:], OP.add, OP.mult)
            hops = ps.tile([128, CH], f32, tag="ps")
            nc.tensor.matmul(hops[:, :128], C["Wn2"][:], zT[:],
                             start=True, stop=True)
            hout = wk.tile([128, 128], f32, tag="hout")
            nc.scalar.activation(hout[:], hops[:, :128], AF.Identity,
                                 bias=C["bn2"][:])
            nc.sync.dma_start(houtT.ap()[w], hout[:])

            # velocity head
            vps = ps.tile([128, CH], f32, tag="ps")
            nc.tensor.matmul(vps[:, :H], C["Wv1"][:],
                             hT_res[:, 128 * w:128 * (w + 1)],
                             start=True, stop=True)
            svT = wk.tile([128, 128], f32, tag="svT")
            if USE_ACT_SILU:
                nc.scalar.activation(svT[:], vps[:, :H], AF.Silu,
                                     bias=C["bv1"][:])
            else:
                sg = wk.tile([128, 128], f32, tag="sgv")
                nc.scalar.activation(sg[:], vps[:, :H], AF.Sigmoid,
                                     bias=C["bv1"][:])
                nc.vector.scalar_tensor_tensor(
                    svT[:], vps[:, :H], C["bv1"][:, :1], sg[:], OP.add, OP.mult)
            vps2 = ps.tile([128, CH], f32, tag="ps")
            nc.tensor.matmul(vps2[:1, :128], C["Wv2"][:], svT[:],
                             start=True, stop=True)
            nc.vector.tensor_scalar(vel_acc[:, 128 * w:128 * (w + 1)],
                                    vps2[:1, :128], C["bv2"][:1, :1], None,
                                    OP.add)

        nc.sync.dma_start(force_o.ap()[:], force_acc[:])
        nc.sync.dma_start(vel_o.ap()[:], vel_acc[:])

    if CAP_WAITS:
        _cap_waits(nc)
    return nc


def _cap_waits(nc, limit=1):
    """Split instructions with >limit sem-waits by hoisting excess waits onto
    inserted NoOps (same engine, immediately preceding) — works around a
    'Too many sync wait commands' walrus codegen limit on this toolchain."""
    import bass_rust
    import concourse.mybir as mybir

    uid = [0]

    def make_nop(engine, waits):
        uid[0] += 1
        nop = bass_rust.InstNoOp(name=f"WCAP-{uid[0]}", ins=[], outs=[])
        nop.engine = engine
        nop.sync_info = mybir.SyncInfo(on_wait=list(waits), on_update=[])
        return nop

    n_split = 0
    for fn in nc.m.functions:
        for bb in fn.blocks:
            out = []
            for ins in bb.instructions:
                si = ins.sync_info
                if si is not None and si.on_wait is not None \
                        and len(si.on_wait) > limit:
                    waits = list(si.on_wait)
                    extra, keep = waits[:-limit], waits[-limit:]
                    for i in range(0, len(extra), limit):
                        out.append(make_nop(ins.engine, extra[i:i + limit]))
                    ins.sync_info = mybir.SyncInfo(
                        on_wait=keep, on_update=list(si.on_update))
                    n_split += 1
                out.append(ins)
            bb.instructions[:] = out
    return n_split


_CACHE = {}


def _get_nc(T):
    if T not in _CACHE:
        _CACHE[T] = _build(T)
    return _CACHE[T]


def _make_in_maps(h, coord_diff, row, col, ws):
    h = np.asarray(h, dtype=np.float32)
    rr_dev, rrT_dev, ci_dev, cd_dev, T = _prep(h, coord_diff, row, col)
    hT_pad = np.zeros((128, NPAD), np.float32)
    hT_pad[:, :N] = h.T
    consts = _weights_np(ws, T)
    in_maps = []
    for c in range(NCORES):
        w0, w1 = c * WPC, (c + 1) * WPC
        m = {
            "hT": hT_pad,
            "hT_own": np.ascontiguousarray(hT_pad[:, w0 * 128:w1 * 128]),
            "rr": np.ascontiguousarray(rr_dev[w0:w1]),
            "rrT": np.ascontiguousarray(rrT_dev[w0:w1]),
            "ci": np.ascontiguousarray(ci_dev[w0:w1]),
            "cdp": np.ascontiguousarray(cd_dev[w0:w1]),
        }
        m.update(consts)
        in_maps.append(m)
    return in_maps, T


# ---------------- entry point ----------------

def kernel(h, coord_diff, row, col,
           We1, be1, We2, be2,
           Wn1, bn1, Wn2, bn2,
           Wc1, bc1, Wc2,
           Wv1, bv1, Wv2, bv2):
    from concourse.bass_utils import run_bass_kernel_spmd

    ws = dict(We1=We1, be1=be1, We2=We2, be2=be2, Wn1=Wn1, bn1=bn1, Wn2=Wn2,
              bn2=bn2, Wc1=Wc1, bc1=bc1, Wc2=Wc2, Wv1=Wv1, bv1=bv1, Wv2=Wv2,
              bv2=bv2)
    in_maps, T = _make_in_maps(h, coord_diff, row, col, ws)
    nc = _get_nc(T)
    res = run_bass_kernel_spmd(nc, in_maps, list(range(NCORES))).results

    vel = np.zeros((NPAD, 1), np.float32)
    force = np.zeros((NPAD, 3), np.float32)
    h_out = np.zeros((NPAD, D), np.float32)
    for c in range(NCORES):
        r = res[c]
        n0 = c * NPC
        vel[n0:n0 + NPC, 0] = r["vel_o"].reshape(-1)
        force[n0:n0 + NPC] = (r["force_o"].reshape(128, WPC, 3)
                              .transpose(1, 0, 2).reshape(NPC, 3))
        h_out[n0:n0 + NPC] = (r["houtT"].reshape(WPC, 128, 128)
                              .transpose(0, 2, 1).reshape(NPC, D))
    return vel[:N], force[:N], h_out[:N]


# revision 27
# speedup vs baseline: 3065.2122x; 3065.2122x over previous
"""EGCL (E(n)-equivariant GNN layer) Trainium2 kernel.

Strategy (8 NeuronCores, data-parallel, no collectives):
  - Sort edges by destination node (row); node space padded to 392 windows of
    128 nodes; each core owns 49 consecutive windows (6272 nodes) and every
    edge landing in them.
  - Separable first layer: e_in @ We1 = p[row] + q[col] + radial*w_r with
    p = h@We1a, q = h@We1b computed per-node on device (phase A); q is stored
    bf16 in DRAM and randomly gathered per 128-edge tile with indirect DMA,
    then PE-transposed into the T-layout accumulation; p[row] is expanded by
    a one-hot matmul (rowrel broadcast via ones-matmul + DVE compare).
  - Edge MLP runs in "T layout" (features on partitions, edges on the free
    axis) so weight matmuls keep weights stationary; per 128-edge tile a
    one-hot matrix (DVE compare vs rowrel) is the stationary operand of the
    segment-sum scatter which accumulates agg/force/cnt in PSUM per window.
  - Node MLP, force normalization and the velocity head run per window on
    resident hT/agg tiles; outputs are written in blocked/transposed layouts
    the host reassembles.
"""

import os
import sys

import numpy as np

for _p in ("/opt/trn_rl_repo",):
    if os.path.isdir(_p) and _p not in sys.path:
        sys.path.insert(0, _p)

import ml_dtypes  # noqa: E402

# ---------------- problem geometry (hardcoded) ----------------
N, E, D, H = 50000, 800000, 128, 128
NCORES = 8
NW = 392                 # total node windows (392*128 = 50176 >= N)
WPC = NW // NCORES       # 49 windows per core
NPC = WPC * 128          # 6272 nodes per core
NPAD = NW * 128          # 50176
NHALF = NPAD // 2        # 25088 (dma_gather int16 index range split)
CH_T = 6                 # tiles (of 128 edges) per pipeline chunk

USE_ACT_SILU = True      # Silu on ACT; False -> Sigmoid + DVE multiply
CAP_WAITS = True         # walrus workaround; CoreSim needs False


# ---------------- host-side preprocessing ----------------

def _prep(h, coord_diff, row, col):
    """Sort/pad edges into window-major device layouts."""
    row = np.asarray(row).astype(np.int64)
    col = np.asarray(col).astype(np.int64)
    cd = np.asarray(coord_diff, dtype=np.float32)

    key = row >> 7
    order = np.argsort(key, kind="stable")
    row_s, col_s, cd_s = row[order], col[order], cd[order]

    counts = np.bincount(key, minlength=NW)
    caph = int(-(-counts.max() // 768) * 768)   # CAP multiple of 768 (CH_T=6)
    cap = caph
    T = cap // 128

    rowrel = np.full((NW, cap), -1.0, np.float32)
    colidx = np.zeros((NW, cap), np.int32)
    cdp = np.zeros((NW, cap, 3), np.float32)
    starts = np.zeros(NW + 1, np.int64)
    starts[1:] = np.cumsum(counts)
    for w in range(NW):
        s, e = starts[w], starts[w + 1]
        n = e - s
        if n == 0:
            continue
        rowrel[w, :n] = (row_s[s:e] - w * 128).astype(np.float32)
        colidx[w, :n] = col_s[s:e].astype(np.int32)
        cdp[w, :n] = cd_s[s:e]

    # device layouts; tile t = pos // 128, partition = pos % 128
    rr_dev = (rowrel.reshape(NW, T, 128).transpose(0, 2, 1)
              .astype(ml_dtypes.bfloat16).copy())                     # [NW,128,T]
    rrT_dev = rowrel.reshape(NW, 1, cap).copy()                       # [NW,1,CAP] f32
    ci_dev = colidx.reshape(NW, T, 128).transpose(0, 2, 1).copy()     # [NW,128,T]
    cd_dev = (cdp.reshape(NW, T, 128, 3)
              .transpose(0, 2, 1, 3).reshape(NW, 128, T * 3).copy())  # f32
    return rr_dev, rrT_dev, ci_dev, cd_dev, T


def _weights_np(ws, T):
    f32 = np.float32
    b16 = ml_dtypes.bfloat16
    We1 = np.asarray(ws["We1"], f32)
    out = {
        "We1a": We1[:128].copy(),
        "We1b": We1[128:256].copy(),
        "wr": We1[256:257].astype(b16),
        "We2": np.asarray(ws["We2"], f32).astype(b16),
        "Wc1": np.asarray(ws["Wc1"], f32).astype(b16),
        "Wc2": np.asarray(ws["Wc2"], f32).astype(b16),
        "Wn1a": np.asarray(ws["Wn1"], f32)[:128].copy(),
        "Wn1b": np.asarray(ws["Wn1"], f32)[128:256].copy(),
        "Wn2": np.asarray(ws["Wn2"], f32),
        "Wv1": np.asarray(ws["Wv1"], f32),
        "Wv2": np.asarray(ws["Wv2"], f32),
        "be1": np.asarray(ws["be1"], f32).reshape(128, 1),
        "be2": np.asarray(ws["be2"], f32).reshape(128, 1),
        "bc1": np.asarray(ws["bc1"], f32).reshape(128, 1),
        "bn1": np.asarray(ws["bn1"], f32).reshape(128, 1),
        "bn2": np.asarray(ws["bn2"], f32).reshape(128, 1),
        "bv1": np.asarray(ws["bv1"], f32).reshape(128, 1),
        "bv2": np.asarray(ws["bv2"], f32).reshape(1, 1),
        "iotat": np.tile(np.arange(128, dtype=f32), (128, T)).astype(b16),
        "iotap": np.arange(128, dtype=f32).reshape(128, 1),
        "ones1": np.ones((1, 128), f32),
        "idf": np.eye(128, dtype=f32),
        "idb": np.eye(128, dtype=f32).astype(b16),
    }
    return out


# ---------------- device program ----------------

def _build(T):
    import concourse.bass as bass
    import concourse.mybir as mybir
    import concourse.tile as tile
    from concourse import library_config
    from concourse.tile_rust import add_dep_helper
    from contextlib import ExitStack

    dt = mybir.dt
    f32, b16, i16 = dt.float32, dt.bfloat16, dt.int16
    AF = mybir.ActivationFunctionType
    OP = mybir.AluOpType

    CAP = T * 128
    CAPH = CAP // 2
    C16 = CAPH // 16
    CH = CH_T * 128                       # chunk edge count (768)
    n_chunks = CAP // CH

    nc = bass.Bass("TRN2", target_bir_lowering=False, debug=False,
                   num_devices=NCORES)

    IN = {}

    def inp(name, shape, dtype):
        IN[name] = nc.dram_tensor(name, list(shape), dtype, kind="ExternalInput")
        return IN[name]

    inp("hT", [128, NPAD], f32)
    inp("hT_own", [128, NPC], f32)
    inp("rr", [WPC, 128, T], b16)
    inp("rrT", [WPC, 1, CAP], f32)
    inp("cdp", [WPC, 128, T * 3], f32)
    inp("ci", [WPC, 128, T], dt.int32)
    inp("We1a", [128, H], f32); inp("We1b", [128, H], f32)
    inp("wr", [1, H], b16)
    inp("We2", [H, H], b16); inp("Wc1", [H, H], b16); inp("Wc2", [H, 1], b16)
    inp("Wn1a", [128, H], f32); inp("Wn1b", [H, H], f32); inp("Wn2", [H, D], f32)
    inp("Wv1", [128, H], f32); inp("Wv2", [H, 1], f32)
    for b in ("be1", "be2", "bc1", "bn1", "bn2", "bv1"):
        inp(b, [128, 1], f32)
    inp("bv2", [1, 1], f32)
    inp("iotat", [128, CAP], b16)
    inp("iotap", [128, 1], f32)
    inp("ones1", [1, 128], f32)
    inp("idf", [128, 128], f32)
    inp("idb", [128, 128], b16)

    houtT = nc.dram_tensor("houtT", [WPC, 128, 128], f32, kind="ExternalOutput")
    force_o = nc.dram_tensor("force_o", [128, WPC * 3], f32, kind="ExternalOutput")
    vel_o = nc.dram_tensor("vel_o", [1, WPC * 128], f32, kind="ExternalOutput")

    qd = nc.dram_tensor("qd", [NPAD, H], b16)

    with tile.TileContext(nc) as tc, ExitStack() as ctx:
        cpool = ctx.enter_context(tc.tile_pool(name="const", bufs=1))
        io = ctx.enter_context(tc.tile_pool(name="io", bufs=4))
        wk = ctx.enter_context(tc.tile_pool(name="wk", bufs=2))
        ps = ctx.enter_context(
            tc.tile_pool(name="ps", bufs=3, space=bass.MemorySpace.PSUM))
        psb = ctx.enter_context(
            tc.tile_pool(name="psb", bufs=1, space=bass.MemorySpace.PSUM))
        psP = ctx.enter_context(
            tc.tile_pool(name="psP", bufs=1, space=bass.MemorySpace.PSUM))

        def mm_split(out, lhsT, rhs, n):
            # matmul in <=512-column pieces so each lands in one PSUM bank
            for s0 in range(0, n, 512):
                s1 = min(s0 + 512, n)
                nc.tensor.matmul(out[:, s0:s1], lhsT, rhs[:, s0:s1],
                                 start=True, stop=True)

        # ---- constants / persistent SBUF ----
        C = {}
        for name in ("We1a", "We1b", "wr", "We2", "Wc1", "Wc2", "Wn1a", "Wn1b",
                     "Wn2", "Wv1", "Wv2", "be1", "be2", "bc1", "bn1", "bn2",
                     "bv1", "bv2", "iotat", "iotap", "ones1", "idf", "idb"):
            t = cpool.tile(list(IN[name].shape), IN[name].dtype, tag=name)
            nc.sync.dma_start(t[:], IN[name].ap())
            C[name] = t

        hT_res = cpool.tile([128, NPC], f32, tag="hT_res")
        p_res = cpool.tile([128, NPC], b16, tag="p_res")
        force_acc = cpool.tile([128, WPC * 3], f32, tag="force_acc")
        vel_acc = cpool.tile([1, WPC * 128], f32, tag="vel_acc")

        nc.sync.dma_start(hT_res[:], IN["hT_own"].ap())

        # ---- phase A: p (bf16, resident) for own windows; q (bf16) in DRAM --
        for w in range(WPC):
            pp = ps.tile([128, CH], f32, tag="ps")
            nc.tensor.matmul(pp[:, :H], hT_res[:, 128 * w:128 * (w + 1)],
                             C["We1a"][:], start=True, stop=True)
            nc.vector.tensor_copy(p_res[:, 128 * w:128 * (w + 1)], pp[:, :H])

        last_q_write = None
        for j in range(NW):
            ht = io.tile([128, 128], f32, tag="ht")
            nc.sync.dma_start(ht[:], IN["hT"].ap()[:, 128 * j:128 * (j + 1)])
            qp = ps.tile([128, CH], f32, tag="ps")
            nc.tensor.matmul(qp[:, :H], ht[:], C["We1b"][:], start=True, stop=True)
            qs = io.tile([128, H], b16, tag="qs")
            nc.vector.tensor_copy(qs[:], qp[:, :H])
            last_q_write = nc.sync.dma_start(
                qd.ap()[128 * j:128 * (j + 1), :], qs[:])

        # ---- phase B/C: per-window edge pipeline + node ops ----
        for w in range(WPC):
            rr_w = wk.tile([128, T], b16, tag="rr")
            nc.sync.dma_start(rr_w[:], IN["rr"].ap()[w])
            cd_w = wk.tile([128, T * 3], f32, tag="cd")
            nc.sync.dma_start(cd_w[:], IN["cdp"].ap()[w])

            rrT_w = wk.tile([1, CAP], f32, tag="rrT")
            nc.sync.dma_start(rrT_w[:], IN["rrT"].ap()[w])
            ci_w = wk.tile([128, T], dt.int32, tag="ci")
            nc.sync.dma_start(ci_w[:], IN["ci"].ap()[w])

            # q[col] gather (e-layout rows, one indirect DMA per 128-edge tile)
            qe_w = wk.tile([128, T * 128], b16, tag="qe")
            for t in range(T):
                g = nc.gpsimd.indirect_dma_start(
                    out=qe_w[:, 128 * t:128 * (t + 1)],
                    out_offset=None,
                    in_=qd.ap(),
                    in_offset=bass.IndirectOffsetOnAxis(
                        ap=ci_w[:, t:t + 1], axis=0),
                )
                if last_q_write is not None:
                    add_dep_helper(g.ins, last_q_write.ins, True,
                                   "gather after q fully written")

            mask_w = wk.tile([128, T], b16, tag="mask")
            nc.vector.tensor_scalar(mask_w[:], rr_w[:], 0.0, None, OP.is_ge)

            # one-hot: ohB[e, n] = (iota_n == rowrel[e])  (single DVE op)
            ohB = wk.tile([128, CAP], b16, tag="ohB")
            nc.vector.scalar_tensor_tensor(
                ohB[:].rearrange("p (t n) -> p t n", n=128),
                C["iotat"][:].rearrange("p (t n) -> p t n", n=128),
                0.0,
                rr_w[:].broadcast_to([128, T, 128]),
                OP.add, OP.is_equal)

            # radial: |cd|^2 -> [1, CAP] bf16 via transpose + sbuf-sbuf dma
            sq_w = wk.tile([128, T * 3], f32, tag="sq")
            nc.vector.tensor_tensor(sq_w[:], cd_w[:], cd_w[:], OP.mult)
            radial_e = wk.tile([128, T], f32, tag="radial")
            nc.vector.tensor_reduce(
                radial_e[:], sq_w[:].rearrange("p (t i) -> p t i", i=3),
                mybir.AxisListType.X, OP.add)
            rT_ps = ps.tile([128, CH], f32, tag="ps")
            nc.tensor.transpose(rT_ps[:T, :128], radial_e[:], C["idf"][:])
            rT_sb = wk.tile([T, 128], b16, tag="rT")
            nc.vector.tensor_copy(rT_sb[:], rT_ps[:T, :128])
            radialT = wk.tile([1, CAP], b16, tag="radialT")
            nc.sync.dma_start(radialT[:], rT_sb[:])

            scat = psP.tile([128, 132], f32, tag="scat")
            z1_w = wk.tile([128, CAP], b16, tag="z1")
            a1T = wk.tile([128, CAP], b16, tag="a1T")
            mT = wk.tile([128, CAP], b16, tag="mT")
            z3_w = wk.tile([128, CAP], b16, tag="z3")
            sT = wk.tile([128, CAP], b16, tag="sT")

            # --- layer 1: z1 = p[row] + q[col] + w_r*radial ; a1 = silu+be1
            for ci in range(n_chunks):
                e0 = ci * CH
                sl = slice(e0, e0 + CH)
                # one-hot A (rowrel broadcast across partitions via ones-matmul)
                rrp = ps.tile([128, CH], f32, tag="ps")
                mm_split(rrp, C["ones1"][:], rrT_w[:, sl], CH)
                ohA = wk.tile([128, CH], b16, tag="ohA")
                nc.vector.tensor_scalar(ohA[:], rrp[:], C["iotap"][:, :1],
                                        None, OP.is_equal)
                m1ps = ps.tile([128, CH], f32, tag="ps")
                for s0 in range(0, CH, 512):
                    s1 = min(s0 + 512, CH)
                    nc.tensor.matmul(m1ps[:, s0:s1], C["wr"][:],
                                     radialT[:, sl][:, s0:s1],
                                     start=True, stop=False)
                    nc.tensor.matmul(m1ps[:, s0:s1],
                                     p_res[:, 128 * w:128 * (w + 1)],
                                     ohA[:, s0:s1], start=False, stop=True)
                qt_ps = psb.tile([128, CH], b16, tag="psb")
                for i in range(CH_T):
                    gt = ci * CH_T + i
                    nc.tensor.transpose(qt_ps[:, 128 * i:128 * (i + 1)],
                                        qe_w[:, 128 * gt:128 * (gt + 1)],
                                        C["idb"][:])
                qt_sb = wk.tile([128, CH], b16, tag="qt")
                nc.vector.tensor_copy(qt_sb[:], qt_ps[:])
                nc.vector.tensor_tensor(z1_w[:, sl], m1ps[:], qt_sb[:], OP.add)
            if USE_ACT_SILU:
                nc.scalar.activation(a1T[:], z1_w[:], AF.Silu, bias=C["be1"][:])
            else:
                sg = wk.tile([128, CAP], b16, tag="sg1")
                nc.scalar.activation(sg[:], z1_w[:], AF.Sigmoid, bias=C["be1"][:])
                nc.vector.scalar_tensor_tensor(
                    a1T[:], z1_w[:], C["be1"][:, :1], sg[:], OP.add, OP.mult)

            # --- layer 2: mT = silu(We2.T@a1 + be2) (chunked from PSUM)
            for ci in range(n_chunks):
                sl = slice(ci * CH, ci * CH + CH)
                m2ps = ps.tile([128, CH], f32, tag="ps")
                mm_split(m2ps, C["We2"][:], a1T[:, sl], CH)
                if USE_ACT_SILU:
                    nc.scalar.activation(mT[:, sl], m2ps[:], AF.Silu,
                                         bias=C["be2"][:])
                else:
                    sg = wk.tile([128, CH], b16, tag="sg2")
                    nc.scalar.activation(sg[:], m2ps[:], AF.Sigmoid,
                                         bias=C["be2"][:])
                    nc.vector.scalar_tensor_tensor(
                        mT[:, sl], m2ps[:], C["be2"][:, :1], sg[:],
                        OP.add, OP.mult)

            # --- coef head: sT = silu(Wc1.T@mT + bc1) (z3 batched)
            for ci in range(n_chunks):
                sl = slice(ci * CH, ci * CH + CH)
                c1ps = ps.tile([128, CH], f32, tag="ps")
                mm_split(c1ps, C["Wc1"][:], mT[:, sl], CH)
                nc.vector.tensor_copy(z3_w[:, sl], c1ps[:])
            if USE_ACT_SILU:
                nc.scalar.activation(sT[:], z3_w[:], AF.Silu, bias=C["bc1"][:])
            else:
                sg = wk.tile([128, CAP], b16, tag="sg3")
                nc.scalar.activation(sg[:], z3_w[:], AF.Sigmoid, bias=C["bc1"][:])
                nc.vector.scalar_tensor_tensor(
                    sT[:], z3_w[:], C["bc1"][:, :1], sg[:], OP.add, OP.mult)

            # --- coef, trans, scatter (per chunk) ---
            for ci in range(n_chunks):
                t0 = ci * CH_T
                cfps = ps.tile([128, CH], f32, tag="ps")
                for i in range(CH_T):
                    gt = t0 + i
                    nc.tensor.matmul(cfps[:, i:i + 1],
                                     sT[:, 128 * gt:128 * (gt + 1)],
                                     C["Wc2"][:], start=True, stop=True)
                coef = wk.tile([128, CH_T], f32, tag="coef")
                nc.vector.tensor_copy(coef[:], cfps[:, :CH_T])

                srhs = wk.tile([128, CH_T * 132], b16, tag="srhs")
                sv = srhs[:].rearrange("p (t c) -> p t c", c=132)
                mtr_ps = psb.tile([128, CH], b16, tag="psb")
                for i in range(CH_T):
                    gt = t0 + i
                    nc.tensor.transpose(mtr_ps[:, 128 * i:128 * (i + 1)],
                                        mT[:, 128 * gt:128 * (gt + 1)],
                                        C["idb"][:])
                nc.vector.tensor_copy(
                    sv[:, :, 0:128],
                    mtr_ps[:].rearrange("p (t e) -> p t e", e=128))
                tmptr = wk.tile([128, CH_T * 3], f32, tag="tmptr")
                nc.vector.tensor_tensor(
                    tmptr[:].rearrange("p (t i) -> p t i", i=3),
                    cd_w[:, 3 * t0:3 * (t0 + CH_T)]
                        .rearrange("p (t i) -> p t i", i=3),
                    coef[:].broadcast_to([128, CH_T, 3]),
                    OP.mult)
                nc.vector.tensor_scalar(
                    sv[:, :, 128:131],
                    tmptr[:].rearrange("p (t i) -> p t i", i=3),
                    -100.0, 100.0, OP.max, OP.min)
                nc.vector.tensor_copy(
                    sv[:, :, 131:132],
                    mask_w[:, t0:t0 + CH_T].broadcast_to([128, CH_T, 1]))

                for i in range(CH_T):
                    gt = t0 + i
                    nc.tensor.matmul(scat[:], ohB[:, 128 * gt:128 * (gt + 1)],
                                     sv[:, i, :], start=(gt == 0),
                                     stop=(gt == T - 1))

            # ---- window epilogue ----
            cntc = wk.tile([128, 1], f32, tag="cntc")
            nc.vector.tensor_scalar(cntc[:], scat[:, 131:132], 1.0, None, OP.max)
            rcp = wk.tile([128, 1], f32, tag="rcp")
            nc.vector.reciprocal(rcp[:], cntc[:])
            nc.vector.tensor_scalar(force_acc[:, 3 * w:3 * (w + 1)],
                                    scat[:, 128:131], rcp[:, :1], None, OP.mult)

            agg_sb = wk.tile([128, 128], f32, tag="agg")
            nc.vector.tensor_copy(agg_sb[:], scat[:, 0:128])
            aggT_ps = ps.tile([128, CH], f32, tag="ps")
            nc.tensor.transpose(aggT_ps[:, :128], agg_sb[:], C["idf"][:])
            aggT = wk.tile([128, 128], f32, tag="aggT")
            nc.vector.tensor_copy(aggT[:], aggT_ps[:, :128])

            # node MLP (T layout): houtT[w] = Wn2.T @ silu(zpre) + bn2
            zps = ps.tile([128, CH], f32, tag="ps")
            nc.tensor.matmul(zps[:, :H], C["Wn1a"][:],
                             hT_res[:, 128 * w:128 * (w + 1)],
                             start=True, stop=False)
            nc.tensor.matmul(zps[:, :H], C["Wn1b"][:], aggT[:],
                             start=False, stop=True)
            zT = wk.tile([128, 128], f32, tag="zT")
            if USE_ACT_SILU:
                nc.scalar.activation(zT[:], zps[:, :H], AF.Silu, bias=C["bn1"][:])
            else:
                sg = wk.tile([128, 128], f32, tag="sgn")
                nc.scalar.activation(sg[:], zps[:, :H], AF.Sigmoid,
                                     bias=C["bn1"][:])
                nc.vector.scalar_tensor_tensor(
                    zT[:], zps[:, :H], C["bn1"][:, :1], sg[:], OP.add, OP.mult)
            hops = ps.tile([128, CH], f32, tag="ps")
            nc.tensor.matmul(hops[:, :128], C["Wn2"][:], zT[:],
                             start=True, stop=True)
            hout = wk.tile([128, 128], f32, tag="hout")
            nc.scalar.activation(hout[:], hops[:, :128], AF.Identity,
                                 bias=C["bn2"][:])
            nc.sync.dma_start(houtT.ap()[w], hout[:])

            # velocity head
            vps = ps.tile([128, CH], f32, tag="ps")
            nc.tensor.matmul(vps[:, :H], C["Wv1"][:],
                             hT_res[:, 128 * w:128 * (w + 1)],
                             start=True, stop=True)
            svT = wk.tile([128, 128], f32, tag="svT")
            if USE_ACT_SILU:
                nc.scalar.activation(svT[:], vps[:, :H], AF.Silu,
                                     bias=C["bv1"][:])
            else:
                sg = wk.tile([128, 128], f32, tag="sgv")
                nc.scalar.activation(sg[:], vps[:, :H], AF.Sigmoid,
                                     bias=C["bv1"][:])
                nc.vector.scalar_tensor_tensor(
                    svT[:], vps[:, :H], C["bv1"][:, :1], sg[:], OP.add, OP.mult)
            vps2 = ps.tile([128, CH], f32, tag="ps")
            nc.tensor.matmul(vps2[:1, :128], C["Wv2"][:], svT[:],
                             start=True, stop=True)
            nc.vector.tensor_scalar(vel_acc[:, 128 * w:128 * (w + 1)],
                                    vps2[:1, :128], C["bv2"][:1, :1], None,
                                    OP.add)

        nc.sync.dma_start(force_o.ap()[:], force_acc[:])
        nc.sync.dma_start(vel_o.ap()[:], vel_acc[:])

    if CAP_WAITS:
        _cap_waits(nc)
    return nc


def _cap_waits(nc, limit=1):
    """Split instructions with >limit sem-waits by hoisting excess waits onto
    inserted NoOps (same engine, immediately preceding) — works around a
    'Too many sync wait commands' walrus codegen limit on this toolchain."""
    import bass_rust
    import concourse.mybir as mybir

    uid = [0]

    def make_nop(engine, waits):
        uid[0] += 1
        nop = bass_rust.InstNoOp(name=f"WCAP-{uid[0]}", ins=[], outs=[])
        nop.engine = engine
        nop.sync_info = mybir.SyncInfo(on_wait=list(waits), on_update=[])
        return nop

    n_split = 0
    for fn in nc.m.functions:
        for bb in fn.blocks:
            out = []
            for ins in bb.instructions:
                si = ins.sync_info
                if si is not None and si.on_wait is not None \
                        and len(si.on_wait) > limit:
                    waits = list(si.on_wait)
                    extra, keep = waits[:-limit], waits[-limit:]
                    for i in range(0, len(extra), limit):
                        out.append(make_nop(ins.engine, extra[i:i + limit]))
                    ins.sync_info = mybir.SyncInfo(
                        on_wait=keep, on_update=list(si.on_update))
                    n_split += 1
                out.append(ins)
            bb.instructions[:] = out
    return n_split


_CACHE = {}


def _get_nc(T):
    if T not in _CACHE:
        _CACHE[T] = _build(T)
    return _CACHE[T]


def _make_in_maps(h, coord_diff, row, col, ws):
    h = np.asarray(h, dtype=np.float32)
    rr_dev, rrT_dev, ci_dev, cd_dev, T = _prep(h, coord_diff, row, col)
    hT_pad = np.zeros((128, NPAD), np.float32)
    hT_pad[:, :N] = h.T
    consts = _weights_np(ws, T)
    in_maps = []
    for c in range(NCORES):
        w0, w1 = c * WPC, (c + 1) * WPC
        m = {
            "hT": hT_pad,
            "hT_own": np.ascontiguousarray(hT_pad[:, w0 * 128:w1 * 128]),
            "rr": np.ascontiguousarray(rr_dev[w0:w1]),
            "rrT": np.ascontiguousarray(rrT_dev[w0:w1]),
            "ci": np.ascontiguousarray(ci_dev[w0:w1]),
            "cdp": np.ascontiguousarray(cd_dev[w0:w1]),
        }
        m.update(consts)
        in_maps.append(m)
    return in_maps, T


# ---------------- entry point ----------------

def kernel(h, coord_diff, row, col,
           We1, be1, We2, be2,
           Wn1, bn1, Wn2, bn2,
           Wc1, bc1, Wc2,
           Wv1, bv1, Wv2, bv2):
    from concourse.bass_utils import run_bass_kernel_spmd

    ws = dict(We1=We1, be1=be1, We2=We2, be2=be2, Wn1=Wn1, bn1=bn1, Wn2=Wn2,
              bn2=bn2, Wc1=Wc1, bc1=bc1, Wc2=Wc2, Wv1=Wv1, bv1=bv1, Wv2=Wv2,
              bv2=bv2)
    in_maps, T = _make_in_maps(h, coord_diff, row, col, ws)
    nc = _get_nc(T)
    res = run_bass_kernel_spmd(nc, in_maps, list(range(NCORES))).results

    vel = np.zeros((NPAD, 1), np.float32)
    force = np.zeros((NPAD, 3), np.float32)
    h_out = np.zeros((NPAD, D), np.float32)
    for c in range(NCORES):
        r = res[c]
        n0 = c * NPC
        vel[n0:n0 + NPC, 0] = r["vel_o"].reshape(-1)
        force[n0:n0 + NPC] = (r["force_o"].reshape(128, WPC, 3)
                              .transpose(1, 0, 2).reshape(NPC, 3))
        h_out[n0:n0 + NPC] = (r["houtT"].reshape(WPC, 128, 128)
                              .transpose(0, 2, 1).reshape(NPC, D))
    return vel[:N], force[:N], h_out[:N]
